# revision 14
# baseline (speedup 1.0000x reference)
"""Multi-head attention on 8 TRN2 NeuronCores.

Sharding: core c -> batch b = c//2, head-group g = c%2 (8 of 16 heads).
Each core computes, for its (batch, 8 heads):
    Q^T/K^T projections (head-dim on partitions) in fp8e4 DoubleRow
    (2 K-tiles per pass, ~2.5x PE throughput vs bf16), V natural layout
    in bf16, transposed scores S^T[t,s] per head (bf16), exp on ACT,
    unnormalized AV^T with a ones-column in V producing the softmax
    denominator row, normalization fused into the PSUM->SBUF copy as a
    DVE divide against a K=1 matmul broadcast of the raw denominator,
    and the partial output projection against this head-group's 512
    rows of Wo (bf16 partial outputs).

Math shortcuts vs the reference (exact in exact arithmetic):
  - bk dropped: scores[s,t] += Q[s].bk is constant over t -> cancels in
    softmax.
  - bv folded into bo on the host: softmax rows sum to 1, so
    AV = A@V0 + bv and the bv term contributes the constant row
    bv_flat @ Wo, added to bo.
  - row masking: masked columns of x_Q^T zeroed on host; bq injected via
    the (pre-scaled) mask row so masked queries get Q=0 -> uniform rows.

fp8 scaling: Wq/Wk are scaled by 2^11 on the host (fp8e4 subnormal
avoidance); Q epilogue multiplies by the host-prescaled mask row
(mask * 2^-11), K epilogue multiplies by 2^-11, so QT/KT are true-scale
bf16 and the exp scale stays 1/8.

Input DMA is spread across the three trigger queues (sync/scalar/
gpsimd) in need-order: Q chunks first (striped), then K, then V, wo
last. The emission does all four Q projections first (K data still in
flight), then pipelines K-proj(p) -> scores of pair p.
"""

import sys

sys.path.insert(0, "/opt/trn_rl_repo")

import numpy as np
import ml_dtypes

B, S, D, H, DH = 4, 1024, 1024, 16, 64
P = 128
NPAIR = 4  # head pairs per core (8 heads)
SW = 2048.0  # fp8 weight scale (2^11)

_CACHED = None


def _build():
    import concourse.bass as bass
    import concourse.mybir as mybir
    from concourse.tile import TileContext

    bf16 = mybir.dt.bfloat16
    f32 = mybir.dt.float32
    fp8 = mybir.dt.float8e4
    DR = mybir.MatmulPerfMode.DoubleRow
    Exp = mybir.ActivationFunctionType.Exp

    nc = bass.Bass()
    xq = nc.dram_tensor("xq", [D, S], fp8, kind="ExternalInput")  # masked cols zeroed
    xk = nc.dram_tensor("xk", [D, S], fp8, kind="ExternalInput")
    xv = nc.dram_tensor("xv", [D, S], bf16, kind="ExternalInput")
    wq = nc.dram_tensor("wq", [D, 512], fp8, kind="ExternalInput")  # x 2^11
    wk = nc.dram_tensor("wk", [D, 512], fp8, kind="ExternalInput")  # x 2^11
    wv = nc.dram_tensor("wv", [D, 512], bf16, kind="ExternalInput")
    wo = nc.dram_tensor("wo", [512, D], bf16, kind="ExternalInput")
    bqc = nc.dram_tensor("bq", [P, 4], f32, kind="ExternalInput")  # x 2^11, pregathered
    mask = nc.dram_tensor("mask", [1, S], bf16, kind="ExternalInput")  # x 2^-11
    out = nc.dram_tensor("out", [S, D], bf16, kind="ExternalOutput")

    with TileContext(nc) as tc:
        with (
            tc.tile_pool(name="persist", bufs=1) as persist,
            tc.tile_pool(name="expp", bufs=4) as expp,
            tc.tile_pool(name="stagep", bufs=4) as stagep,
            tc.tile_pool(name="outp", bufs=2) as outp,
            tc.tile_pool(name="ps", bufs=3, space="PSUM") as psp,
            tc.tile_pool(name="ps2", bufs=2, space="PSUM") as psp2,
            tc.tile_pool(name="ps3", bufs=1, space="PSUM") as psp3,
        ):
            def ps_tile():
                return psp.tile([P, 512], f32, tag="ps", name="ps")

            def sc_tile():
                return psp2.tile([P, 1024], f32, tag="sc", name="sc")

            # ---- constants and small rows (gpsimd queue; tiny) ----
            mask_sb = persist.tile([1, S], bf16, tag="mask")
            nc.gpsimd.dma_start(mask_sb[:], mask[:])
            ones_sb = persist.tile([1, 512], bf16, tag="ones")
            nc.vector.memset(ones_sb[:], 1.0)
            bqc_sb = persist.tile([P, 4], f32, tag="bqc")
            nc.gpsimd.dma_start(bqc_sb[:], bqc[:])
            mask_bc = persist.tile([P, S], bf16, tag="mask_bc")

            xq_sb = persist.tile([P, 8, S], fp8, tag="xq")
            xk_sb = persist.tile([P, 8, S], fp8, tag="xk")
            xv_sb = persist.tile([P, 8, S], bf16, tag="xv")
            wq_sb = persist.tile([P, 8, 512], fp8, tag="wq")
            wk_sb = persist.tile([P, 8, 512], fp8, tag="wk")
            wv_sb = persist.tile([P, 8, 512], bf16, tag="wv")
            wo_sb = persist.tile([P, 4, D], bf16, tag="wo_sb")
            xq_r = xq.rearrange("(c p) s -> p c s", p=P)
            xk_r = xk.rearrange("(c p) s -> p c s", p=P)
            xv_r = xv.rearrange("(c p) s -> p c s", p=P)
            wq_r = wq.rearrange("(c p) m -> p c m", p=P)
            wk_r = wk.rearrange("(c p) m -> p c m", p=P)
            wv_r = wv.rearrange("(c p) m -> p c m", p=P)
            # Inputs spread over the three DMA trigger queues in need-order:
            # all Q chunks first (striped), then K, then V; wo last.
            qs = (nc.sync, nc.scalar, nc.gpsimd)
            for dc in range(8):
                qs[dc % 3].dma_start(wq_sb[:, dc, :], wq_r[:, dc, :])
                qs[(dc + 1) % 3].dma_start(xq_sb[:, dc, :], xq_r[:, dc, :])
            for dc in range(8):
                qs[dc % 3].dma_start(wk_sb[:, dc, :], wk_r[:, dc, :])
                qs[(dc + 1) % 3].dma_start(xk_sb[:, dc, :], xk_r[:, dc, :])
            for dc in range(8):
                qs[dc % 3].dma_start(wv_sb[:, dc, :], wv_r[:, dc, :])
                qs[(dc + 1) % 3].dma_start(xv_sb[:, dc, :], xv_r[:, dc, :])
            nc.gpsimd.dma_start(wo_sb[:], wo.rearrange("(c p) m -> p c m", p=P))

            QT = [persist.tile([P, S], bf16, tag=f"qt{p}", name=f"qt{p}") for p in range(NPAIR)]
            KT = [persist.tile([P, S], bf16, tag=f"kt{p}", name=f"kt{p}") for p in range(NPAIR)]
            AVT = [persist.tile([P, S], bf16, tag=f"avt{p}", name=f"avt{p}") for p in range(NPAIR)]
            vaug = persist.tile([P, 8, 8 * 65], bf16, tag="vaug")
            nc.vector.memset(
                vaug.rearrange("p t (h x) -> p t h x", x=65)[:, :, :, 64:65], 1.0
            )

            expS = [None] * 8
            stages = [None] * 8
            # denominator rows repacked [s] -> [128 partitions, 8/partition]
            # so the reciprocal runs partition-parallel, then scattered back
            # to [1, 8, S] rows for the K=1 broadcast matmuls.
            packed = persist.tile([P, 8, 8], f32, tag="packed")
            packed_b = persist.tile([P, 8, 8], bf16, tag="packedb")
            rdrow = persist.tile([1, 8, S], bf16, tag="rdrow")

            def emit_proj(p, w_sb, x_sb, masked):
                # fp8e4 DoubleRow: 2 K-tiles per pass, 4 passes cover K=1024
                for st in range(2):
                    ps = ps_tile()
                    for dci in range(4):
                        nc.tensor.matmul(
                            ps[:],
                            lhsT=w_sb[:, 2 * dci : 2 * dci + 2, p * P : (p + 1) * P],
                            rhs=x_sb[:, 2 * dci : 2 * dci + 2, st * 512 : (st + 1) * 512],
                            start=(dci == 0),
                            stop=(dci == 3),
                            perf_mode=DR,
                        )
                    dst = (QT if masked else KT)[p][:, st * 512 : (st + 1) * 512]
                    if masked:
                        # (ps + bq*SW) * (mask/SW) -> true-scale Q
                        nc.vector.scalar_tensor_tensor(
                            dst,
                            ps[:],
                            bqc_sb[:, p : p + 1],
                            mask_bc[:, st * 512 : (st + 1) * 512],
                            mybir.AluOpType.add,
                            mybir.AluOpType.mult,
                        )
                    else:
                        nc.vector.tensor_scalar_mul(dst, ps[:], 1.0 / SW)

            def emit_v(tcn):
                ps = ps_tile()
                for dc in range(8):
                    nc.tensor.matmul(
                        ps[:],
                        lhsT=xv_sb[:, dc, tcn * P : (tcn + 1) * P],
                        rhs=wv_sb[:, dc, 0:512],
                        start=(dc == 0),
                        stop=(dc == 7),
                    )
                nc.vector.tensor_copy(
                    vaug[:, tcn, :].rearrange("p (h x) -> p h x", x=65)[:, :, 0:64],
                    ps[:].rearrange("p (h v) -> p h v", v=64),
                )

            def emit_scores(h):
                p, base = h // 2, 64 * (h % 2)
                expS[h] = expp.tile([P, 8, S], bf16, tag="expS", name="expS")
                for tcn in range(8):
                    ps = sc_tile()
                    for st in range(2):
                        nc.tensor.matmul(
                            ps[:, st * 512 : (st + 1) * 512],
                            lhsT=KT[p][base : base + 64, tcn * P : (tcn + 1) * P],
                            rhs=QT[p][base : base + 64, st * 512 : (st + 1) * 512],
                            start=True,
                            stop=True,
                        )
                    nc.scalar.activation(expS[h][:, tcn, :], ps[:], Exp, scale=0.125)

            def emit_uav(h):
                # unnormalized AV^T + the denominator row (65th lhsT column)
                p, base = h // 2, 64 * (h % 2)
                stage = stagep.tile([1, S], f32, tag="stage", name="stage")
                stages[h] = stage
                for st in range(2):
                    psu = ps_tile()
                    for tcn in range(8):
                        nc.tensor.matmul(
                            psu[0:65, :],
                            lhsT=vaug[:, tcn, h * 65 : (h + 1) * 65],
                            rhs=expS[h][:, tcn, st * 512 : (st + 1) * 512],
                            start=(tcn == 0),
                            stop=(tcn == 7),
                        )
                    nc.vector.tensor_copy(
                        AVT[p][base : base + 64, st * 512 : (st + 1) * 512],
                        psu[0:64, :],
                    )
                    nc.vector.tensor_copy(
                        stage[0:1, st * 512 : (st + 1) * 512], psu[64:65, :]
                    )

            def emit_recip(h):
                # partition-parallel reciprocal: gather the [1,S] row into
                # [128,8], recip, scatter back as a bf16 row (gpsimd queue;
                # latency hidden behind ~2 heads of PE work)
                qeng = (nc.sync, nc.scalar, nc.gpsimd)[h % 3]
                qeng.dma_start(
                    packed[:, h, :],
                    stages[h][0:1, :].rearrange("o (p j) -> o p j", j=8),
                )
                nc.vector.reciprocal(packed[:, h, :], packed[:, h, :])
                nc.vector.tensor_copy(packed_b[:, h, :], packed[:, h, :])
                qeng.dma_start(
                    rdrow[0:1, h, :].rearrange("o (p j) -> o p j", j=8),
                    packed_b[:, h, :],
                )

            def emit_norm(h):
                # AVT[h] *= 1/denom[h,s], broadcast across the 64 v-partitions
                # via a K=1 outer-product matmul of the reciprocal row.
                p, base = h // 2, 64 * (h % 2)
                for st in range(2):
                    psr = psp3.tile([P, 512], f32, tag="psr", name="psr")
                    nc.tensor.matmul(
                        psr[0:64, :],
                        lhsT=ones_sb[0:1, 0:64],
                        rhs=rdrow[0:1, h, st * 512 : (st + 1) * 512],
                        start=True,
                        stop=True,
                    )
                    av = AVT[p][base : base + 64, st * 512 : (st + 1) * 512]
                    nc.vector.tensor_mul(av, av, psr[0:64, :])

            def emit_out(sc):
                osb = outp.tile([P, D], bf16, tag="osb", name="osb")
                ps0, ps1 = ps_tile(), ps_tile()
                for p in range(NPAIR):
                    for mt, ps in ((0, ps0), (1, ps1)):
                        nc.tensor.matmul(
                            ps[:],
                            lhsT=AVT[p][:, sc * P : (sc + 1) * P],
                            rhs=wo_sb[:, p, mt * 512 : (mt + 1) * 512],
                            start=(p == 0),
                            stop=(p == NPAIR - 1),
                        )
                nc.vector.tensor_copy(osb[:, 0:512], ps0[:])
                nc.vector.tensor_copy(osb[:, 512:1024], ps1[:])
                eng = (nc.sync, nc.scalar, nc.gpsimd)[sc % 3]
                eng.dma_start(out[sc * P : (sc + 1) * P, :], osb[:])

            # PE warmup: dummy matmuls on a memset tile keep the array busy
            # while inputs stream in, so the clock is ramped for real work.
            warm = persist.tile([P, 512], bf16, tag="warm")
            nc.vector.memset(warm[:], 0.0)
            for _ in range(12):
                psw = ps_tile()
                nc.tensor.matmul(
                    psw[:],
                    lhsT=warm[:, 0:P],
                    rhs=warm[:],
                    start=True,
                    stop=True,
                )

            # mask broadcast [128, S] for the fused Q bias+mask epilogue
            for st in range(2):
                psm = ps_tile()
                nc.tensor.matmul(
                    psm[:],
                    lhsT=ones_sb[0:1, 0:P],
                    rhs=mask_sb[0:1, st * 512 : (st + 1) * 512],
                    start=True,
                    stop=True,
                )
                nc.vector.tensor_copy(mask_bc[:, st * 512 : (st + 1) * 512], psm[:])

            # ---- software-pipelined emission ----
            for p in range(NPAIR):
                emit_proj(p, wq_sb, xq_sb, True)
            emit_proj(0, wk_sb, xk_sb, False)
            emit_scores(0)
            emit_proj(1, wk_sb, xk_sb, False)
            emit_scores(1)
            emit_proj(2, wk_sb, xk_sb, False)
            emit_scores(2)
            emit_proj(3, wk_sb, xk_sb, False)
            emit_scores(3)
            for tcn in range(8):
                emit_v(tcn)
            emit_uav(0)
            emit_recip(0)
            emit_scores(4)
            emit_uav(1)
            emit_recip(1)
            emit_scores(5)
            emit_uav(2)
            emit_recip(2)
            emit_norm(0)
            emit_scores(6)
            emit_uav(3)
            emit_recip(3)
            emit_norm(1)
            emit_scores(7)
            emit_uav(4)
            emit_recip(4)
            emit_norm(2)
            emit_uav(5)
            emit_recip(5)
            emit_norm(3)
            emit_uav(6)
            emit_recip(6)
            emit_norm(4)
            emit_norm(5)
            emit_uav(7)
            emit_recip(7)
            emit_norm(6)
            emit_norm(7)
            for sc in range(8):
                emit_out(sc)

    _split_multiwait(nc)
    return nc


def _split_multiwait(nc):
    """This container's walrus rejects >1 sync wait on CTRL-class
    instructions (Tile's exit Drain carries one per outstanding proc).
    Hoist all but the last wait onto preceding same-engine NoOps."""
    import concourse.mybir as mybir

    for f in nc.m.functions:
        for bb in f.blocks:
            insts = list(bb.instructions)
            res, changed = [], False
            for inst in insts:
                si = inst.sync_info
                waits = list(si.on_wait) if si is not None else []
                if len(waits) > 1:
                    for w in waits[:-1]:
                        res.append(
                            mybir.InstNoOp(
                                name=nc.get_next_instruction_name(),
                                sync_info=mybir.SyncInfo(on_wait=[w], on_update=[]),
                                bass_nofuse=True,
                                engine=inst.engine,
                            )
                        )
                    inst.sync_info = mybir.SyncInfo(
                        on_wait=[waits[-1]], on_update=list(si.on_update)
                    )
                    changed = True
                res.append(inst)
            if changed:
                bb.instructions = res


def _shard_inputs(x_Q, x_K, x_V, src_batch_lens, Wq, bq, Wk, bk, Wv, bv, Wo, bo):
    bf = ml_dtypes.bfloat16
    f8 = ml_dtypes.float8_e4m3
    f32 = np.float32
    in_maps = []
    # head-major packed weights [D, H*DH] and bias [1, H*DH]
    wq_all = (np.asarray(Wq, f32).transpose(1, 0, 2).reshape(D, H * DH) * SW).astype(f8)
    wk_all = (np.asarray(Wk, f32).transpose(1, 0, 2).reshape(D, H * DH) * SW).astype(f8)
    wv_all = np.asarray(Wv, f32).transpose(1, 0, 2).reshape(D, H * DH).astype(bf)
    bq_all = (np.asarray(bq, f32).reshape(1, H * DH) * SW).astype(f32)
    wo_bf = np.asarray(Wo, f32).astype(bf)
    for c in range(8):
        b, g = c // 2, c % 2
        ln = int(src_batch_lens[b])
        m = (np.arange(S) < ln).astype(f32)
        xqT = np.ascontiguousarray(np.asarray(x_Q[b], f32).T * m[None, :]).astype(f8)
        xkT = np.ascontiguousarray(np.asarray(x_K[b], f32).T).astype(f8)
        xvT = np.ascontiguousarray(np.asarray(x_V[b], f32).T).astype(bf)
        hs = slice(g * 512, (g + 1) * 512)
        in_maps.append(
            {
                "xq": xqT,
                "xk": xkT,
                "xv": xvT,
                "wq": np.ascontiguousarray(wq_all[:, hs]),
                "wk": np.ascontiguousarray(wk_all[:, hs]),
                "wv": np.ascontiguousarray(wv_all[:, hs]),
                "wo": np.ascontiguousarray(wo_bf[hs, :]),
                "bq": np.ascontiguousarray(bq_all[0, hs].reshape(4, P).T),
                "mask": (m / SW).reshape(1, S).astype(bf),
            }
        )
    return in_maps


def kernel(**inputs):
    global _CACHED
    from concourse.bass_utils import run_bass_kernel_spmd

    if _CACHED is None:
        _CACHED = _build()
    nc = _CACHED
    in_maps = _shard_inputs(**inputs)
    res = run_bass_kernel_spmd(nc, in_maps, core_ids=list(range(8)))
    # bv folded: softmax rows sum to 1 so AV = A@V0 + bv; bv contributes
    # the constant row bv_flat @ Wo.
    bo2 = np.asarray(inputs["bo"], np.float32) + np.asarray(
        inputs["bv"], np.float32
    ).reshape(-1) @ np.asarray(inputs["Wo"], np.float32)
    out = np.empty((B, S, D), np.float32)
    for b in range(B):
        out[b] = (
            res.results[2 * b]["out"].astype(np.float32)
            + res.results[2 * b + 1]["out"].astype(np.float32)
            + bo2[None, :]
        )
    return out


# revision 15
# speedup vs baseline: 1.0614x; 1.0614x over previous
"""Multi-head attention on 8 TRN2 NeuronCores.

Sharding: core c -> batch b = c//2, head-group g = c%2 (8 of 16 heads).
Each core computes, for its (batch, 8 heads):
    Q^T/K^T projections (head-dim on partitions) in fp8e4 DoubleRow
    (2 K-tiles per pass, ~2.5x PE throughput vs bf16), V natural layout
    in bf16, transposed scores S^T[t,s] per head (bf16), exp on ACT,
    unnormalized AV^T with a ones-column in V producing the softmax
    denominator row, normalization fused into the PSUM->SBUF copy as a
    DVE divide against a K=1 matmul broadcast of the raw denominator,
    and the partial output projection against this head-group's 512
    rows of Wo (bf16 partial outputs).

Math shortcuts vs the reference (exact in exact arithmetic):
  - bk dropped: scores[s,t] += Q[s].bk is constant over t -> cancels in
    softmax.
  - bv folded into bo on the host: softmax rows sum to 1, so
    AV = A@V0 + bv and the bv term contributes the constant row
    bv_flat @ Wo, added to bo.
  - row masking: masked columns of x_Q^T zeroed on host; bq injected via
    the (pre-scaled) mask row so masked queries get Q=0 -> uniform rows.

fp8 scaling: Wq/Wk are scaled by 2^11 on the host (fp8e4 subnormal
avoidance); Q epilogue multiplies by the host-prescaled mask row
(mask * 2^-11), K epilogue multiplies by 2^-11, so QT/KT are true-scale
bf16 and the exp scale stays 1/8.

Input DMA is spread across the three trigger queues (sync/scalar/
gpsimd) in need-order: Q chunks first (striped), then K, then V, wo
last. The emission does all four Q projections first (K data still in
flight), then pipelines K-proj(p) -> scores of pair p.
"""

import sys

sys.path.insert(0, "/opt/trn_rl_repo")

import numpy as np
import ml_dtypes

B, S, D, H, DH = 4, 1024, 1024, 16, 64
P = 128
NPAIR = 4  # head pairs per core (8 heads)
SW = 2048.0  # fp8 weight scale (2^11)

_CACHED = None


def _build():
    import concourse.bass as bass
    import concourse.mybir as mybir
    from concourse.tile import TileContext

    bf16 = mybir.dt.bfloat16
    f32 = mybir.dt.float32
    fp8 = mybir.dt.float8e4
    DR = mybir.MatmulPerfMode.DoubleRow
    Exp = mybir.ActivationFunctionType.Exp

    nc = bass.Bass()
    xq = nc.dram_tensor("xq", [D, S], fp8, kind="ExternalInput")  # masked cols zeroed
    xk = nc.dram_tensor("xk", [D, S], fp8, kind="ExternalInput")
    xv = nc.dram_tensor("xv", [D, S], bf16, kind="ExternalInput")
    wq = nc.dram_tensor("wq", [D, 512], fp8, kind="ExternalInput")  # x 2^11
    wk = nc.dram_tensor("wk", [D, 512], fp8, kind="ExternalInput")  # x 2^11
    wv = nc.dram_tensor("wv", [D, 512], bf16, kind="ExternalInput")
    wo = nc.dram_tensor("wo", [512, D], bf16, kind="ExternalInput")
    bqc = nc.dram_tensor("bq", [P, 4], f32, kind="ExternalInput")  # x 2^11, pregathered
    mask = nc.dram_tensor("mask", [1, S], bf16, kind="ExternalInput")  # x 2^-11
    out = nc.dram_tensor("out", [S, D], bf16, kind="ExternalOutput")

    with TileContext(nc) as tc:
        with (
            tc.tile_pool(name="persist", bufs=1) as persist,
            tc.tile_pool(name="expp", bufs=3) as expp,
            tc.tile_pool(name="stagep", bufs=4) as stagep,
            tc.tile_pool(name="outp", bufs=3) as outp,
            tc.tile_pool(name="ps", bufs=3, space="PSUM") as psp,
            tc.tile_pool(name="ps2", bufs=2, space="PSUM") as psp2,
            tc.tile_pool(name="ps3", bufs=1, space="PSUM") as psp3,
        ):
            def ps_tile():
                return psp.tile([P, 512], f32, tag="ps", name="ps")

            def sc_tile():
                return psp2.tile([P, 1024], f32, tag="sc", name="sc")

            # ---- constants and small rows (gpsimd queue; tiny) ----
            mask_sb = persist.tile([1, S], bf16, tag="mask")
            nc.gpsimd.dma_start(mask_sb[:], mask[:])
            ones_sb = persist.tile([1, 512], bf16, tag="ones")
            nc.vector.memset(ones_sb[:], 1.0)
            bqc_sb = persist.tile([P, 4], f32, tag="bqc")
            nc.gpsimd.dma_start(bqc_sb[:], bqc[:])
            mask_bc = persist.tile([P, S], bf16, tag="mask_bc")

            xq_sb = persist.tile([P, 8, S], fp8, tag="xq")
            xk_sb = persist.tile([P, 8, S], fp8, tag="xk")
            xv_sb = persist.tile([P, 8, S], bf16, tag="xv")
            wq_sb = persist.tile([P, 8, 512], fp8, tag="wq")
            wk_sb = persist.tile([P, 8, 512], fp8, tag="wk")
            wv_sb = persist.tile([P, 8, 512], bf16, tag="wv")
            wo_sb = persist.tile([P, 4, D], bf16, tag="wo_sb")
            xq_r = xq.rearrange("(c p) s -> p c s", p=P)
            xk_r = xk.rearrange("(c p) s -> p c s", p=P)
            xv_r = xv.rearrange("(c p) s -> p c s", p=P)
            wq_r = wq.rearrange("(c p) m -> p c m", p=P)
            wk_r = wk.rearrange("(c p) m -> p c m", p=P)
            wv_r = wv.rearrange("(c p) m -> p c m", p=P)
            # Inputs spread over the three DMA trigger queues in need-order:
            # all Q chunks first (striped), then K, then V; wo last.
            qs = (nc.sync, nc.scalar, nc.gpsimd)
            for dc in range(8):
                qs[dc % 3].dma_start(wq_sb[:, dc, :], wq_r[:, dc, :])
                qs[(dc + 1) % 3].dma_start(xq_sb[:, dc, :], xq_r[:, dc, :])
            for dc in range(8):
                qs[dc % 3].dma_start(wk_sb[:, dc, :], wk_r[:, dc, :])
                qs[(dc + 1) % 3].dma_start(xk_sb[:, dc, :], xk_r[:, dc, :])
            for dc in range(8):
                qs[dc % 3].dma_start(wv_sb[:, dc, :], wv_r[:, dc, :])
                qs[(dc + 1) % 3].dma_start(xv_sb[:, dc, :], xv_r[:, dc, :])
            nc.gpsimd.dma_start(wo_sb[:], wo.rearrange("(c p) m -> p c m", p=P))

            QT = [persist.tile([P, S], bf16, tag=f"qt{p}", name=f"qt{p}") for p in range(NPAIR)]
            KT = [persist.tile([P, S], bf16, tag=f"kt{p}", name=f"kt{p}") for p in range(NPAIR)]
            AVT = [persist.tile([P, S], bf16, tag=f"avt{p}", name=f"avt{p}") for p in range(NPAIR)]
            vaug = persist.tile([P, 8, 8 * 65], bf16, tag="vaug")
            nc.vector.memset(
                vaug.rearrange("p t (h x) -> p t h x", x=65)[:, :, :, 64:65], 1.0
            )

            expS = [None] * 8
            stages = [None] * 8
            # denominator rows repacked [s] -> [128 partitions, 8/partition]
            # so the reciprocal runs partition-parallel, then scattered back
            # to [1, 8, S] rows for the K=1 broadcast matmuls.
            packed = persist.tile([P, 8, 8], f32, tag="packed")
            packed_b = persist.tile([P, 8, 8], bf16, tag="packedb")
            rdrow = persist.tile([1, 8, S], bf16, tag="rdrow")

            def emit_proj(p, w_sb, x_sb, masked):
                # fp8e4 DoubleRow: 2 K-tiles per pass, 4 passes cover K=1024
                for st in range(2):
                    ps = ps_tile()
                    for dci in range(4):
                        nc.tensor.matmul(
                            ps[:],
                            lhsT=w_sb[:, 2 * dci : 2 * dci + 2, p * P : (p + 1) * P],
                            rhs=x_sb[:, 2 * dci : 2 * dci + 2, st * 512 : (st + 1) * 512],
                            start=(dci == 0),
                            stop=(dci == 3),
                            perf_mode=DR,
                        )
                    dst = (QT if masked else KT)[p][:, st * 512 : (st + 1) * 512]
                    if masked:
                        # (ps + bq*SW) * (mask/SW) -> true-scale Q
                        nc.vector.scalar_tensor_tensor(
                            dst,
                            ps[:],
                            bqc_sb[:, p : p + 1],
                            mask_bc[:, st * 512 : (st + 1) * 512],
                            mybir.AluOpType.add,
                            mybir.AluOpType.mult,
                        )
                    else:
                        nc.vector.tensor_scalar_mul(dst, ps[:], 1.0 / SW)

            def emit_v(tcn):
                ps = ps_tile()
                for dc in range(8):
                    nc.tensor.matmul(
                        ps[:],
                        lhsT=xv_sb[:, dc, tcn * P : (tcn + 1) * P],
                        rhs=wv_sb[:, dc, 0:512],
                        start=(dc == 0),
                        stop=(dc == 7),
                    )
                nc.vector.tensor_copy(
                    vaug[:, tcn, :].rearrange("p (h x) -> p h x", x=65)[:, :, 0:64],
                    ps[:].rearrange("p (h v) -> p h v", v=64),
                )

            def emit_scores(h):
                p, base = h // 2, 64 * (h % 2)
                expS[h] = expp.tile([P, 8, S], bf16, tag="expS", name="expS")
                for tcn in range(8):
                    ps = sc_tile()
                    for st in range(2):
                        nc.tensor.matmul(
                            ps[:, st * 512 : (st + 1) * 512],
                            lhsT=KT[p][base : base + 64, tcn * P : (tcn + 1) * P],
                            rhs=QT[p][base : base + 64, st * 512 : (st + 1) * 512],
                            start=True,
                            stop=True,
                        )
                    nc.scalar.activation(expS[h][:, tcn, :], ps[:], Exp, scale=0.125)

            def emit_uav(h):
                # unnormalized AV^T + the denominator row (65th lhsT column)
                p, base = h // 2, 64 * (h % 2)
                stage = stagep.tile([1, S], f32, tag="stage", name="stage")
                stages[h] = stage
                for st in range(2):
                    psu = ps_tile()
                    for tcn in range(8):
                        nc.tensor.matmul(
                            psu[0:65, :],
                            lhsT=vaug[:, tcn, h * 65 : (h + 1) * 65],
                            rhs=expS[h][:, tcn, st * 512 : (st + 1) * 512],
                            start=(tcn == 0),
                            stop=(tcn == 7),
                        )
                    nc.vector.tensor_copy(
                        AVT[p][base : base + 64, st * 512 : (st + 1) * 512],
                        psu[0:64, :],
                    )
                    nc.vector.tensor_copy(
                        stage[0:1, st * 512 : (st + 1) * 512], psu[64:65, :]
                    )

            def emit_recip_act(h):
                # tail heads: 1/denom = exp(-ln(denom)) on the (by now idle)
                # ACT engine -- skips the pack/scatter DMA latency
                lnrow = stagep.tile([1, S], f32, tag="stage", name="lnrow")
                nc.scalar.activation(
                    lnrow[:], stages[h][:], mybir.ActivationFunctionType.Ln
                )
                nc.scalar.activation(
                    rdrow[0:1, h, :], lnrow[:],
                    mybir.ActivationFunctionType.Exp, scale=-1.0,
                )

            def emit_recip(h):
                # partition-parallel reciprocal: gather the [1,S] row into
                # [128,8], recip, scatter back as a bf16 row (gpsimd queue;
                # latency hidden behind ~2 heads of PE work)
                nc.sync.dma_start(
                    packed[:, h, :],
                    stages[h][0:1, :].rearrange("o (p j) -> o p j", j=8),
                )
                nc.vector.reciprocal(packed[:, h, :], packed[:, h, :])
                nc.vector.tensor_copy(packed_b[:, h, :], packed[:, h, :])
                nc.sync.dma_start(
                    rdrow[0:1, h, :].rearrange("o (p j) -> o p j", j=8),
                    packed_b[:, h, :],
                )

            def emit_norm(h):
                # AVT[h] *= 1/denom[h,s], broadcast across the 64 v-partitions
                # via a K=1 outer-product matmul of the reciprocal row.
                p, base = h // 2, 64 * (h % 2)
                for st in range(2):
                    psr = psp3.tile([P, 512], f32, tag="psr", name="psr")
                    nc.tensor.matmul(
                        psr[0:64, :],
                        lhsT=ones_sb[0:1, 0:64],
                        rhs=rdrow[0:1, h, st * 512 : (st + 1) * 512],
                        start=True,
                        stop=True,
                    )
                    av = AVT[p][base : base + 64, st * 512 : (st + 1) * 512]
                    nc.vector.tensor_mul(av, av, psr[0:64, :])

            def emit_out(sc):
                osb = outp.tile([P, D], bf16, tag="osb", name="osb")
                ps0, ps1 = ps_tile(), ps_tile()
                for p in range(NPAIR):
                    for mt, ps in ((0, ps0), (1, ps1)):
                        nc.tensor.matmul(
                            ps[:],
                            lhsT=AVT[p][:, sc * P : (sc + 1) * P],
                            rhs=wo_sb[:, p, mt * 512 : (mt + 1) * 512],
                            start=(p == 0),
                            stop=(p == NPAIR - 1),
                        )
                nc.vector.tensor_copy(osb[:, 0:512], ps0[:])
                nc.vector.tensor_copy(osb[:, 512:1024], ps1[:])
                eng = (nc.sync, nc.scalar, nc.gpsimd)[sc % 3]
                eng.dma_start(out[sc * P : (sc + 1) * P, :], osb[:])

            # PE warmup: dummy matmuls on a memset tile keep the array busy
            # while inputs stream in, so the clock is ramped for real work.
            warm = persist.tile([P, 512], bf16, tag="warm")
            nc.vector.memset(warm[:], 0.0)
            for _ in range(12):
                psw = ps_tile()
                nc.tensor.matmul(
                    psw[:],
                    lhsT=warm[:, 0:P],
                    rhs=warm[:],
                    start=True,
                    stop=True,
                )

            # mask broadcast [128, S] for the fused Q bias+mask epilogue
            for st in range(2):
                psm = ps_tile()
                nc.tensor.matmul(
                    psm[:],
                    lhsT=ones_sb[0:1, 0:P],
                    rhs=mask_sb[0:1, st * 512 : (st + 1) * 512],
                    start=True,
                    stop=True,
                )
                nc.vector.tensor_copy(mask_bc[:, st * 512 : (st + 1) * 512], psm[:])

            # ---- software-pipelined emission ----
            for p in range(NPAIR):
                emit_proj(p, wq_sb, xq_sb, True)
            emit_proj(0, wk_sb, xk_sb, False)
            emit_scores(0)
            emit_proj(1, wk_sb, xk_sb, False)
            emit_scores(1)
            emit_proj(2, wk_sb, xk_sb, False)
            emit_scores(2)
            emit_proj(3, wk_sb, xk_sb, False)
            for tcn in range(8):
                emit_v(tcn)
            emit_uav(0)
            emit_recip(0)
            emit_scores(3)
            emit_uav(1)
            emit_recip(1)
            emit_scores(4)
            emit_uav(2)
            emit_recip(2)
            emit_norm(0)
            emit_scores(5)
            emit_uav(3)
            emit_recip(3)
            emit_norm(1)
            emit_scores(6)
            emit_uav(4)
            emit_recip(4)
            emit_norm(2)
            emit_scores(7)
            emit_uav(5)
            emit_recip(5)
            emit_norm(3)
            emit_uav(6)
            emit_recip_act(6)
            emit_norm(4)
            emit_norm(5)
            emit_uav(7)
            emit_recip_act(7)
            emit_norm(6)
            emit_norm(7)
            for sc in range(8):
                emit_out(sc)

    _split_multiwait(nc)
    return nc


def _split_multiwait(nc):
    """This container's walrus rejects >1 sync wait on CTRL-class
    instructions (Tile's exit Drain carries one per outstanding proc).
    Hoist all but the last wait onto preceding same-engine NoOps."""
    import concourse.mybir as mybir

    for f in nc.m.functions:
        for bb in f.blocks:
            insts = list(bb.instructions)
            res, changed = [], False
            for inst in insts:
                si = inst.sync_info
                waits = list(si.on_wait) if si is not None else []
                if len(waits) > 1:
                    for w in waits[:-1]:
                        res.append(
                            mybir.InstNoOp(
                                name=nc.get_next_instruction_name(),
                                sync_info=mybir.SyncInfo(on_wait=[w], on_update=[]),
                                bass_nofuse=True,
                                engine=inst.engine,
                            )
                        )
                    inst.sync_info = mybir.SyncInfo(
                        on_wait=[waits[-1]], on_update=list(si.on_update)
                    )
                    changed = True
                res.append(inst)
            if changed:
                bb.instructions = res


def _shard_inputs(x_Q, x_K, x_V, src_batch_lens, Wq, bq, Wk, bk, Wv, bv, Wo, bo):
    bf = ml_dtypes.bfloat16
    f8 = ml_dtypes.float8_e4m3
    f32 = np.float32
    in_maps = []
    # head-major packed weights [D, H*DH] and bias [1, H*DH]
    wq_all = (np.asarray(Wq, f32).transpose(1, 0, 2).reshape(D, H * DH) * SW).astype(f8)
    wk_all = (np.asarray(Wk, f32).transpose(1, 0, 2).reshape(D, H * DH) * SW).astype(f8)
    wv_all = np.asarray(Wv, f32).transpose(1, 0, 2).reshape(D, H * DH).astype(bf)
    bq_all = (np.asarray(bq, f32).reshape(1, H * DH) * SW).astype(f32)
    wo_bf = np.asarray(Wo, f32).astype(bf)
    for c in range(8):
        b, g = c // 2, c % 2
        ln = int(src_batch_lens[b])
        m = (np.arange(S) < ln).astype(f32)
        xqT = np.ascontiguousarray(np.asarray(x_Q[b], f32).T * m[None, :]).astype(f8)
        xkT = np.ascontiguousarray(np.asarray(x_K[b], f32).T).astype(f8)
        xvT = np.ascontiguousarray(np.asarray(x_V[b], f32).T).astype(bf)
        hs = slice(g * 512, (g + 1) * 512)
        in_maps.append(
            {
                "xq": xqT,
                "xk": xkT,
                "xv": xvT,
                "wq": np.ascontiguousarray(wq_all[:, hs]),
                "wk": np.ascontiguousarray(wk_all[:, hs]),
                "wv": np.ascontiguousarray(wv_all[:, hs]),
                "wo": np.ascontiguousarray(wo_bf[hs, :]),
                "bq": np.ascontiguousarray(bq_all[0, hs].reshape(4, P).T),
                "mask": (m / SW).reshape(1, S).astype(bf),
            }
        )
    return in_maps


def kernel(**inputs):
    global _CACHED
    from concourse.bass_utils import run_bass_kernel_spmd

    if _CACHED is None:
        _CACHED = _build()
    nc = _CACHED
    in_maps = _shard_inputs(**inputs)
    res = run_bass_kernel_spmd(nc, in_maps, core_ids=list(range(8)))
    # bv folded: softmax rows sum to 1 so AV = A@V0 + bv; bv contributes
    # the constant row bv_flat @ Wo.
    bo2 = np.asarray(inputs["bo"], np.float32) + np.asarray(
        inputs["bv"], np.float32
    ).reshape(-1) @ np.asarray(inputs["Wo"], np.float32)
    out = np.empty((B, S, D), np.float32)
    for b in range(B):
        out[b] = (
            res.results[2 * b]["out"].astype(np.float32)
            + res.results[2 * b + 1]["out"].astype(np.float32)
            + bo2[None, :]
        )
    return out
